# revision 24
# baseline (speedup 1.0000x reference)
"""KimiMoEGate on 8 Trainium2 NeuronCores.

Data-parallel over tokens: each core takes 1024 tokens (8 blocks of 128),
the full gate weight, and produces (topk_idx, topk_weight) for its shard.

Precision scheme (logit rms err ~1e-5 vs fp64, ~3 mismatched tokens of 8192):
  logits = [xh.wh]_f16  +  2^-17 * ([x8.wl8]_f8dr + [xl8.w8]_f8dr)
where xh = f16(x), xl8 = f8((x - xh)*2^11), x8 = f8(xh) (derived on device
by the ACT engine), wh = f16(w), wl8 = f8((w - wh)*2^17), w8 = f8(w*2^6).
The f8 correction passes use DoubleRow perf mode (2 k-tiles contracted per
instruction), so a block costs 56 + 28 + 28 = 112 matmul issues.

Host uploads 3 bytes/elem of x (f16 hi + f8 lo) in PE-ready layout
(partition-major, contiguous per-partition lines), no on-device split.

Routing epilogue per 128-token block (all DVE):
  grouped top-2 via max + match_replace + max; top-4 groups via max8 +
  is_ge threshold; top-8 experts via max8/max_index; weights recovered
  order-exactly via an 8x8 index-equality match against uncorrected scores.
"""
import sys
sys.path.insert(0, '/opt/trn_rl_repo')
import numpy as np
import ml_dtypes
import concourse.bass as bass
from concourse import bacc
import concourse.mybir as mybir
from concourse.bass_utils import run_bass_kernel_spmd
from concourse.tile import TileContext

F32 = mybir.dt.float32
F16 = mybir.dt.float16
F8 = mybir.dt.float8e4
U32 = mybir.dt.uint32
I32 = mybir.dt.int32
AX = mybir.AxisListType
ALU = mybir.AluOpType
ACTF = mybir.ActivationFunctionType
DR = mybir.MatmulPerfMode.DoubleRow

T, H, E = 8192, 7168, 256
NCORES = 8
TPC = T // NCORES            # 1024 tokens per core
KT = H // 128                # 56 contraction tiles
NB = TPC // 128              # 8 blocks of 128 tokens
NEG = -1e30
SW = 2.0 ** 17               # correction-pass scale

_cache = {}


def _ensure_ntff_hook():
    """Register the axon NTFF profiling hook if the boot shim didn't.

    The image's `antenv` stub lacks `axon_hooks`, so trn_boot's hook
    registration silently no-ops and `trace=True` crashes on import.
    Recreate the tiny module and install the ctypes-based hook.
    """
    try:
        import antenv.axon_hooks  # noqa: F401
        return
    except ImportError:
        pass
    import types
    try:
        import antenv
    except ImportError:
        return
    mod = types.ModuleType("antenv.axon_hooks")
    mod._hook = None
    mod.set_axon_ntff_profile_hook = lambda h: setattr(mod, "_hook", h)
    mod.get_axon_ntff_profile_hook = lambda: mod._hook
    sys.modules["antenv.axon_hooks"] = mod
    antenv.axon_hooks = mod
    try:
        from trn_agent_boot.trn_boot import _ntff_profile_via_ctypes
        hook = _ntff_profile_via_ctypes('/opt/axon/libaxon_pjrt.so')
        if hook is not None:
            mod._hook = hook
    except Exception:
        pass


_ensure_ntff_hook()


def _build():
    if "nc" in _cache:
        return _cache["nc"]
    nc = bacc.Bacc("TRN2", target_bir_lowering=False, debug=False,
                   num_devices=NCORES)
    xh_d = nc.dram_tensor("xh", [NB, 128, KT, 128], F16, kind="ExternalInput")
    xl8_d = nc.dram_tensor("xl8", [NB, 128, KT, 128], F8, kind="ExternalInput")
    wh_d = nc.dram_tensor("wh", [128, KT, E], F16, kind="ExternalInput")
    wl8_d = nc.dram_tensor("wl8", [128, KT, E], F8, kind="ExternalInput")
    bias = nc.dram_tensor("bias", [E], F32, kind="ExternalInput")
    # outputs partition-major [128, NB, 8]; host transposes to [TPC, 8]
    o_idx = nc.dram_tensor("o_idx", [128, NB, 8], I32, kind="ExternalOutput")
    o_w = nc.dram_tensor("o_w", [128, NB, 8], F32, kind="ExternalOutput")

    with TileContext(nc) as tc:
        with (
            tc.tile_pool(name="wpool", bufs=1) as wpool,
            tc.tile_pool(name="xpool", bufs=3) as xpool,
            tc.tile_pool(name="small", bufs=2) as small,
            tc.tile_pool(name="ps", bufs=2, space="PSUM") as ps,
        ):
            # ---- DMA issue order matters: the first block's main pass
            # needs only wh chunk 0 + xh[0], so those go first; the fp8
            # correction tensors land while the main pass runs ----
            wh = wpool.tile([128, KT, E], F16)

            def load_x(b, nch=1):
                xh = xpool.tile([128, KT, 128], F16, tag="xh")
                xl8 = xpool.tile([128, KT, 128], F8, tag="xl8")
                step = KT // nch
                for k0 in range(0, KT, step):
                    nc.sync.dma_start(xh[:, k0:k0 + step],
                                      xh_d[b, :, k0:k0 + step])
                nc.sync.dma_start(xl8[:], xl8_d[b])
                return xh, xl8

            def convert(xh):
                x8 = xpool.tile([128, KT, 128], F8, tag="x8")
                nc.scalar.activation(x8[:], xh[:], ACTF.Copy)
                return x8

            # startup: wh streams on the ACT dma queue while xh0 streams on
            # the SP queue, interleaved at 14-k-tile granularity to match
            # the main pass's k-ordered consumption. w8 = f8(wh*64) is
            # derived on the (idle) DVE instead of being uploaded, saving
            # 1.8 MB of critical startup DMA.
            xh0 = xpool.tile([128, KT, 128], F16, tag="xh")
            xl80 = xpool.tile([128, KT, 128], F8, tag="xl8")
            w8 = wpool.tile([128, KT, E], F8)
            wl8 = wpool.tile([128, KT, E], F8)
            chunks = [(0, 7), (7, 7), (14, 14), (28, 14), (42, 14)]
            for k0, n in chunks:
                nc.scalar.dma_start(wh[:, k0:k0 + n], wh_d[:, k0:k0 + n])
                nc.sync.dma_start(xh0[:, k0:k0 + n], xh_d[0, :, k0:k0 + n])
            nc.scalar.dma_start(xl80[:], xl8_d[0])
            for k0, n in chunks:
                nc.vector.tensor_scalar(w8[:, k0:k0 + n], wh[:, k0:k0 + n],
                                        64.0, None, op0=ALU.mult)
            nc.scalar.dma_start(wl8[:], wl8_d[:])
            blk0 = (xh0, xl80)
            blk1 = load_x(1, nch=2)
            bias_rep = wpool.tile([128, E], F32)
            nc.scalar.dma_start(bias_rep[:],
                                bias[None, :].to_broadcast([128, E]))
            acc_w = wpool.tile([128, NB, 8], F32)
            acc_i = wpool.tile([128, NB, 8], U32)

            # software pipeline: x DMA runs 2 blocks ahead, the ACT f16->f8
            # conversion 1 block ahead, so neither gates the PE stream
            pend = [blk0, blk1]
            x8_pend = [convert(blk0[0])]

            for b in range(NB):
                xh, xl8 = pend.pop(0)
                x8 = x8_pend.pop(0)
                if b + 2 < NB:
                    pend.append(load_x(b + 2))

                # ---- GEMM: f16 main + two f8 DoubleRow corrections ----
                psA = ps.tile([128, E], F32, tag="psA")
                psB = ps.tile([128, E], F32, tag="psB")
                for k in range(KT):
                    nc.tensor.matmul(psA[:], xh[:, k], wh[:, k],
                                     start=(k == 0), stop=(k == KT - 1))
                # xl8.w8 first: it does not depend on the ACT-derived x8,
                # and w8 is scheduled to land before wl8 at startup
                for j in range(KT // 2):
                    nc.tensor.matmul(psB[:], xl8[:, 2 * j:2 * j + 2],
                                     w8[:, 2 * j:2 * j + 2],
                                     start=(j == 0), stop=False, perf_mode=DR)
                for j in range(KT // 2):
                    nc.tensor.matmul(psB[:], x8[:, 2 * j:2 * j + 2],
                                     wl8[:, 2 * j:2 * j + 2],
                                     start=False, stop=(j == KT // 2 - 1),
                                     perf_mode=DR)

                # ---- epilogue: logits, sigmoid, routing ----
                # ACT order: logA (ready at main-pass end, during the corr
                # passes) -> next block's x8 conversion -> sigmoid
                logA = small.tile([128, E], F32, tag="logA")
                nc.scalar.activation(logA[:], psA[:], ACTF.Copy)
                if b + 1 < NB:
                    x8_pend.append(convert(pend[0][0]))
                logits = small.tile([128, E], F32, tag="logits")
                nc.vector.scalar_tensor_tensor(logits[:], psB[:], 1.0 / SW,
                                               logA[:], op0=ALU.mult,
                                               op1=ALU.add)
                s = small.tile([128, E], F32, tag="s")
                nc.scalar.activation(s[:], logits[:], ACTF.Sigmoid)
                sc = small.tile([128, E], F32, tag="sc")
                nc.vector.tensor_tensor(sc[:], s[:], bias_rep[:], ALU.add)

                scg = sc[:].rearrange("p (g e) -> p g e", g=8)
                gm = small.tile([128, 8], F32, tag="gm")
                nc.vector.tensor_reduce(gm[:], scg, AX.X, ALU.max)
                scr = small.tile([128, E], F32, tag="scr")
                nc.vector.match_replace(scr[:], gm[:], sc[:], NEG)
                gm2 = small.tile([128, 8], F32, tag="gm2")
                nc.vector.tensor_reduce(
                    gm2[:], scr[:].rearrange("p (g e) -> p g e", g=8),
                    AX.X, ALU.max)
                gsum = small.tile([128, 8], F32, tag="gsum")
                nc.vector.tensor_tensor(gsum[:], gm[:], gm2[:], ALU.add)
                g8 = small.tile([128, 8], F32, tag="g8")
                nc.vector.max(g8[:], gsum[:])
                gmask = small.tile([128, 8], F32, tag="gmask")
                nc.vector.tensor_scalar(gmask[:], gsum[:], g8[:, 3:4], None,
                                        op0=ALU.is_ge)
                tmp = small.tile([128, E], F32, tag="tmp")
                nc.vector.tensor_tensor(
                    tmp[:].rearrange("p (g e) -> p g e", g=8), scg,
                    gmask[:, :, None].to_broadcast([128, 8, 32]), ALU.mult)
                v8 = small.tile([128, 8], F32, tag="v8")
                nc.vector.max(v8[:], tmp[:])
                i8 = acc_i[:, b, :]
                nc.vector.max_index(i8, v8[:], tmp[:])

                # select the top-8 positions by thresholding at the 8th
                # value (ties are measure-zero for random inputs), fused
                # with the uncorrected-score masking in one DVE op
                s_sel = small.tile([128, E], F32, tag="s_sel")
                nc.vector.scalar_tensor_tensor(s_sel[:], tmp[:], v8[:, 7:8],
                                               s[:], op0=ALU.is_ge,
                                               op1=ALU.mult)
                w8s = small.tile([128, 8], F32, tag="w8s")
                nc.vector.max(w8s[:], s_sel[:])
                is8 = small.tile([128, 8], U32, tag="is8")
                nc.vector.max_index(is8[:], w8s[:], s_sel[:])
                ssum = small.tile([128, 1], F32, tag="ssum")
                nc.vector.tensor_reduce(ssum[:], w8s[:], AX.X, ALU.add)
                rec = small.tile([128, 1], F32, tag="rec")
                nc.vector.reciprocal(rec[:], ssum[:])

                eq = small.tile([128, 8, 8], F32, tag="eq")
                nc.vector.tensor_tensor(
                    eq[:],
                    is8[:, None, :].to_broadcast([128, 8, 8]),
                    acc_i[:, b, :, None].to_broadcast([128, 8, 8]),
                    ALU.is_equal)
                prod = small.tile([128, 8, 8], F32, tag="prod")
                nc.vector.tensor_tensor(
                    prod[:], eq[:],
                    w8s[:, None, :].to_broadcast([128, 8, 8]), ALU.mult)
                w8t = small.tile([128, 8], F32, tag="w8t")
                nc.vector.tensor_reduce(w8t[:], prod[:], AX.X, ALU.add)
                nc.vector.tensor_scalar(acc_w[:, b, :], w8t[:], rec[:], 2.5,
                                        op0=ALU.mult, op1=ALU.mult)
            nc.sync.dma_start(o_w[:], acc_w[:])
            nc.sync.dma_start(o_idx[:], acc_i[:].bitcast(I32))
    nc.compile()
    _cache["nc"] = nc
    return nc


def kernel(hidden_states, weight, e_score_correction_bias):
    nc = _build()
    x = np.asarray(hidden_states, dtype=np.float32)
    w = np.asarray(weight, dtype=np.float32)
    b = np.asarray(e_score_correction_bias, dtype=np.float32)

    # ---- host-side precision split + PE-ready layout ----
    wh = w.astype(np.float16)
    wl = (w - wh.astype(np.float32)) * np.float32(SW)
    wl8_np = wl.astype(ml_dtypes.float8_e4m3)

    def lay_w(a):  # [E, H] -> [128p, KT, E]
        return np.ascontiguousarray(a.T.reshape(KT, 128, E).transpose(1, 0, 2))

    wh_l = lay_w(wh)
    wl8_l = lay_w(wl8_np)

    xh = x.astype(np.float16)
    xl8 = ((x - xh.astype(np.float32)) * np.float32(2048.0)).astype(
        ml_dtypes.float8_e4m3)

    def lay_x(a, c):  # core shard [TPC, H] -> [NB, 128p, KT, 128n]
        sh = a[c * TPC:(c + 1) * TPC]
        t = sh.T.reshape(KT, 128, NB, 128).transpose(2, 1, 0, 3)
        return np.ascontiguousarray(t)

    in_maps = []
    for c in range(NCORES):
        in_maps.append({"xh": lay_x(xh, c), "xl8": lay_x(xl8, c),
                        "wh": wh_l, "wl8": wl8_l, "bias": b})

    full = run_bass_kernel_spmd(nc, in_maps, list(range(NCORES)))
    _cache["last"] = full
    res = full.results
    # device outputs are [128, NB, 8] partition-major; token t = b*128 + p
    idx = np.concatenate(
        [res[c]["o_idx"].transpose(1, 0, 2).reshape(TPC, 8)
         for c in range(NCORES)], axis=0)
    wgt = np.concatenate(
        [res[c]["o_w"].transpose(1, 0, 2).reshape(TPC, 8)
         for c in range(NCORES)], axis=0)
    return idx.astype(np.int32), wgt.astype(np.float32)


# revision 25
# speedup vs baseline: 1.0339x; 1.0339x over previous
"""KimiMoEGate on 8 Trainium2 NeuronCores.

Data-parallel over tokens: each core takes 1024 tokens (8 blocks of 128),
the full gate weight, and produces (topk_idx, topk_weight) for its shard.

Precision scheme (logit rms err ~1e-5 vs fp64, ~3 mismatched tokens of 8192):
  logits = [xh.wh]_f16  +  2^-17 * ([x8.wl8]_f8dr + [xl8.w8]_f8dr)
where xh = f16(x), xl8 = f8((x - xh)*2^11), x8 = f8(xh) (derived on device
by the ACT engine), wh = f16(w), wl8 = f8((w - wh)*2^17), w8 = f8(w*2^6).
The f8 correction passes use DoubleRow perf mode (2 k-tiles contracted per
instruction), so a block costs 56 + 28 + 28 = 112 matmul issues.

Host uploads 3 bytes/elem of x (f16 hi + f8 lo) in PE-ready layout
(partition-major, contiguous per-partition lines), no on-device split.

Routing epilogue per 128-token block (all DVE):
  grouped top-2 via max + match_replace + max; top-4 groups via max8 +
  is_ge threshold; top-8 experts via max8/max_index; weights recovered
  order-exactly via an 8x8 index-equality match against uncorrected scores.
"""
import sys
sys.path.insert(0, '/opt/trn_rl_repo')
import numpy as np
import ml_dtypes
import concourse.bass as bass
from concourse import bacc
import concourse.mybir as mybir
from concourse.bass_utils import run_bass_kernel_spmd
from concourse.tile import TileContext

F32 = mybir.dt.float32
F16 = mybir.dt.float16
F8 = mybir.dt.float8e4
U32 = mybir.dt.uint32
I32 = mybir.dt.int32
AX = mybir.AxisListType
ALU = mybir.AluOpType
ACTF = mybir.ActivationFunctionType
DR = mybir.MatmulPerfMode.DoubleRow

T, H, E = 8192, 7168, 256
NCORES = 8
TPC = T // NCORES            # 1024 tokens per core
KT = H // 128                # 56 contraction tiles
NB = TPC // 128              # 8 blocks of 128 tokens
NEG = -1e30
SW = 2.0 ** 17               # correction-pass scale

_cache = {}


def _ensure_ntff_hook():
    """Register the axon NTFF profiling hook if the boot shim didn't.

    The image's `antenv` stub lacks `axon_hooks`, so trn_boot's hook
    registration silently no-ops and `trace=True` crashes on import.
    Recreate the tiny module and install the ctypes-based hook.
    """
    try:
        import antenv.axon_hooks  # noqa: F401
        return
    except ImportError:
        pass
    import types
    try:
        import antenv
    except ImportError:
        return
    mod = types.ModuleType("antenv.axon_hooks")
    mod._hook = None
    mod.set_axon_ntff_profile_hook = lambda h: setattr(mod, "_hook", h)
    mod.get_axon_ntff_profile_hook = lambda: mod._hook
    sys.modules["antenv.axon_hooks"] = mod
    antenv.axon_hooks = mod
    try:
        from trn_agent_boot.trn_boot import _ntff_profile_via_ctypes
        hook = _ntff_profile_via_ctypes('/opt/axon/libaxon_pjrt.so')
        if hook is not None:
            mod._hook = hook
    except Exception:
        pass


_ensure_ntff_hook()


def _build():
    if "nc" in _cache:
        return _cache["nc"]
    nc = bacc.Bacc("TRN2", target_bir_lowering=False, debug=False,
                   num_devices=NCORES)
    xh_d = nc.dram_tensor("xh", [NB, 128, KT, 128], F16, kind="ExternalInput")
    xl8_d = nc.dram_tensor("xl8", [NB, 128, KT, 128], F8, kind="ExternalInput")
    wh_d = nc.dram_tensor("wh", [128, KT, E], F16, kind="ExternalInput")
    wl8_d = nc.dram_tensor("wl8", [128, KT, E], F8, kind="ExternalInput")
    bias = nc.dram_tensor("bias", [E], F32, kind="ExternalInput")
    # outputs partition-major [128, NB, 8]; host transposes to [TPC, 8]
    o_idx = nc.dram_tensor("o_idx", [128, NB, 8], I32, kind="ExternalOutput")
    o_w = nc.dram_tensor("o_w", [128, NB, 8], F32, kind="ExternalOutput")

    with TileContext(nc) as tc:
        with (
            tc.tile_pool(name="wpool", bufs=1) as wpool,
            tc.tile_pool(name="xpool", bufs=3) as xpool,
            tc.tile_pool(name="small", bufs=2) as small,
            tc.tile_pool(name="ps", bufs=2, space="PSUM") as ps,
        ):
            # ---- DMA issue order matters: the first block's main pass
            # needs only wh chunk 0 + xh[0], so those go first; the fp8
            # correction tensors land while the main pass runs ----
            wh = wpool.tile([128, KT, E], F16)

            def load_x(b, nch=1):
                xh = xpool.tile([128, KT, 128], F16, tag="xh")
                xl8 = xpool.tile([128, KT, 128], F8, tag="xl8")
                step = KT // nch
                for k0 in range(0, KT, step):
                    nc.sync.dma_start(xh[:, k0:k0 + step],
                                      xh_d[b, :, k0:k0 + step])
                nc.sync.dma_start(xl8[:], xl8_d[b])
                return xh, xl8

            def convert(xh):
                x8 = xpool.tile([128, KT, 128], F8, tag="x8")
                nc.scalar.activation(x8[:], xh[:], ACTF.Copy)
                return x8

            # startup: wh streams on the ACT dma queue while xh0 streams on
            # the SP queue, interleaved at 14-k-tile granularity to match
            # the main pass's k-ordered consumption. w8 = f8(wh*64) is
            # derived on the (idle) DVE instead of being uploaded, saving
            # 1.8 MB of critical startup DMA.
            xh0 = xpool.tile([128, KT, 128], F16, tag="xh")
            xl80 = xpool.tile([128, KT, 128], F8, tag="xl8")
            w8 = wpool.tile([128, KT, E], F8)
            wl8 = wpool.tile([128, KT, E], F8)
            chunks = [(0, 7), (7, 7), (14, 14), (28, 14), (42, 14)]
            for k0, n in chunks:
                nc.scalar.dma_start(wh[:, k0:k0 + n], wh_d[:, k0:k0 + n])
                nc.sync.dma_start(xh0[:, k0:k0 + n], xh_d[0, :, k0:k0 + n])
            nc.sync.dma_start(xl80[:], xl8_d[0])
            for k0, n in chunks:
                nc.vector.tensor_scalar(w8[:, k0:k0 + n], wh[:, k0:k0 + n],
                                        64.0, None, op0=ALU.mult)
            blk0 = (xh0, xl80)
            # conv(0) must be the next ACT instruction after the wh issues:
            # every extra DMA_DIRECT2D on the ACT queue delays x8(0) and
            # with it block 0's second correction pass
            x8_pend = [convert(blk0[0])]
            nc.scalar.dma_start(wl8[:], wl8_d[:])
            blk1 = load_x(1, nch=2)
            bias_rep = wpool.tile([128, E], F32)
            nc.scalar.dma_start(bias_rep[:],
                                bias[None, :].to_broadcast([128, E]))
            acc_w = wpool.tile([128, NB, 8], F32)
            acc_i = wpool.tile([128, NB, 8], U32)

            # software pipeline: x DMA runs 2 blocks ahead, the ACT f16->f8
            # conversion 1 block ahead, so neither gates the PE stream
            pend = [blk0, blk1]

            for b in range(NB):
                xh, xl8 = pend.pop(0)
                x8 = x8_pend.pop(0)
                if b + 2 < NB:
                    pend.append(load_x(b + 2))

                # ---- GEMM: f16 main + two f8 DoubleRow corrections ----
                psA = ps.tile([128, E], F32, tag="psA")
                psB = ps.tile([128, E], F32, tag="psB")
                for k in range(KT):
                    nc.tensor.matmul(psA[:], xh[:, k], wh[:, k],
                                     start=(k == 0), stop=(k == KT - 1))
                # xl8.w8 first: it does not depend on the ACT-derived x8,
                # and w8 is scheduled to land before wl8 at startup
                for j in range(KT // 2):
                    nc.tensor.matmul(psB[:], xl8[:, 2 * j:2 * j + 2],
                                     w8[:, 2 * j:2 * j + 2],
                                     start=(j == 0), stop=False, perf_mode=DR)
                for j in range(KT // 2):
                    nc.tensor.matmul(psB[:], x8[:, 2 * j:2 * j + 2],
                                     wl8[:, 2 * j:2 * j + 2],
                                     start=False, stop=(j == KT // 2 - 1),
                                     perf_mode=DR)

                # ---- epilogue: logits, sigmoid, routing ----
                # ACT order: logA (ready at main-pass end, during the corr
                # passes) -> next block's x8 conversion -> sigmoid
                logA = small.tile([128, E], F32, tag="logA")
                nc.scalar.activation(logA[:], psA[:], ACTF.Copy)
                if b + 1 < NB:
                    x8_pend.append(convert(pend[0][0]))
                logits = small.tile([128, E], F32, tag="logits")
                nc.vector.scalar_tensor_tensor(logits[:], psB[:], 1.0 / SW,
                                               logA[:], op0=ALU.mult,
                                               op1=ALU.add)
                s = small.tile([128, E], F32, tag="s")
                nc.scalar.activation(s[:], logits[:], ACTF.Sigmoid)
                sc = small.tile([128, E], F32, tag="sc")
                nc.vector.tensor_tensor(sc[:], s[:], bias_rep[:], ALU.add)

                scg = sc[:].rearrange("p (g e) -> p g e", g=8)
                gm = small.tile([128, 8], F32, tag="gm")
                nc.vector.tensor_reduce(gm[:], scg, AX.X, ALU.max)
                scr = small.tile([128, E], F32, tag="scr")
                nc.vector.match_replace(scr[:], gm[:], sc[:], NEG)
                gm2 = small.tile([128, 8], F32, tag="gm2")
                nc.vector.tensor_reduce(
                    gm2[:], scr[:].rearrange("p (g e) -> p g e", g=8),
                    AX.X, ALU.max)
                gsum = small.tile([128, 8], F32, tag="gsum")
                nc.vector.tensor_tensor(gsum[:], gm[:], gm2[:], ALU.add)
                g8 = small.tile([128, 8], F32, tag="g8")
                nc.vector.max(g8[:], gsum[:])
                gmask = small.tile([128, 8], F32, tag="gmask")
                nc.vector.tensor_scalar(gmask[:], gsum[:], g8[:, 3:4], None,
                                        op0=ALU.is_ge)
                tmp = small.tile([128, E], F32, tag="tmp")
                nc.vector.tensor_tensor(
                    tmp[:].rearrange("p (g e) -> p g e", g=8), scg,
                    gmask[:, :, None].to_broadcast([128, 8, 32]), ALU.mult)
                v8 = small.tile([128, 8], F32, tag="v8")
                nc.vector.max(v8[:], tmp[:])
                i8 = acc_i[:, b, :]
                nc.vector.max_index(i8, v8[:], tmp[:])

                # select the top-8 positions by thresholding at the 8th
                # value (ties are measure-zero for random inputs), fused
                # with the uncorrected-score masking in one DVE op
                s_sel = small.tile([128, E], F32, tag="s_sel")
                nc.vector.scalar_tensor_tensor(s_sel[:], tmp[:], v8[:, 7:8],
                                               s[:], op0=ALU.is_ge,
                                               op1=ALU.mult)
                w8s = small.tile([128, 8], F32, tag="w8s")
                nc.vector.max(w8s[:], s_sel[:])
                is8 = small.tile([128, 8], U32, tag="is8")
                nc.vector.max_index(is8[:], w8s[:], s_sel[:])
                ssum = small.tile([128, 1], F32, tag="ssum")
                nc.vector.tensor_reduce(ssum[:], w8s[:], AX.X, ALU.add)
                rec = small.tile([128, 1], F32, tag="rec")
                nc.vector.reciprocal(rec[:], ssum[:])

                eq = small.tile([128, 8, 8], F32, tag="eq")
                nc.vector.tensor_tensor(
                    eq[:],
                    is8[:, None, :].to_broadcast([128, 8, 8]),
                    acc_i[:, b, :, None].to_broadcast([128, 8, 8]),
                    ALU.is_equal)
                prod = small.tile([128, 8, 8], F32, tag="prod")
                nc.vector.tensor_tensor(
                    prod[:], eq[:],
                    w8s[:, None, :].to_broadcast([128, 8, 8]), ALU.mult)
                w8t = small.tile([128, 8], F32, tag="w8t")
                nc.vector.tensor_reduce(w8t[:], prod[:], AX.X, ALU.add)
                nc.vector.tensor_scalar(acc_w[:, b, :], w8t[:], rec[:], 2.5,
                                        op0=ALU.mult, op1=ALU.mult)
            nc.sync.dma_start(o_w[:], acc_w[:])
            nc.sync.dma_start(o_idx[:], acc_i[:].bitcast(I32))
    nc.compile()
    _cache["nc"] = nc
    return nc


def kernel(hidden_states, weight, e_score_correction_bias):
    nc = _build()
    x = np.asarray(hidden_states, dtype=np.float32)
    w = np.asarray(weight, dtype=np.float32)
    b = np.asarray(e_score_correction_bias, dtype=np.float32)

    # ---- host-side precision split + PE-ready layout ----
    wh = w.astype(np.float16)
    wl = (w - wh.astype(np.float32)) * np.float32(SW)
    wl8_np = wl.astype(ml_dtypes.float8_e4m3)

    def lay_w(a):  # [E, H] -> [128p, KT, E]
        return np.ascontiguousarray(a.T.reshape(KT, 128, E).transpose(1, 0, 2))

    wh_l = lay_w(wh)
    wl8_l = lay_w(wl8_np)

    xh = x.astype(np.float16)
    xl8 = ((x - xh.astype(np.float32)) * np.float32(2048.0)).astype(
        ml_dtypes.float8_e4m3)

    def lay_x(a, c):  # core shard [TPC, H] -> [NB, 128p, KT, 128n]
        sh = a[c * TPC:(c + 1) * TPC]
        t = sh.T.reshape(KT, 128, NB, 128).transpose(2, 1, 0, 3)
        return np.ascontiguousarray(t)

    in_maps = []
    for c in range(NCORES):
        in_maps.append({"xh": lay_x(xh, c), "xl8": lay_x(xl8, c),
                        "wh": wh_l, "wl8": wl8_l, "bias": b})

    full = run_bass_kernel_spmd(nc, in_maps, list(range(NCORES)))
    _cache["last"] = full
    res = full.results
    # device outputs are [128, NB, 8] partition-major; token t = b*128 + p
    idx = np.concatenate(
        [res[c]["o_idx"].transpose(1, 0, 2).reshape(TPC, 8)
         for c in range(NCORES)], axis=0)
    wgt = np.concatenate(
        [res[c]["o_w"].transpose(1, 0, 2).reshape(TPC, 8)
         for c in range(NCORES)], axis=0)
    return idx.astype(np.int32), wgt.astype(np.float32)
